# revision 33
# baseline (speedup 1.0000x reference)
"""Trainium2 Bass kernel for nn_BoneRefusion (17-group BoneMLP over [B,T,16,3]).

Strategy (pure data parallel over batch, 8 cores):
  - Host pre-packs per-core inputs feature-major in a 2-set layout:
      x2 [98, S] bf16, S = tokens_per_core/2.
      Rows 0-47 = 48 features (16 bones x 3 coords) of token set A (first
      half), row 48 = ones (bakes b1 into layer 1), rows 49-96 = set B,
      row 97 = ones. Column j holds the token pair (A_j, B_j).
  - Layer 1 (h = relu(x @ W1 + b1)): 17 column-unit matmuls of 32 PE columns
    each (4 passes x 4 units + one unit for group 16's hidden).
  - Layer 2 (out = h @ W2 + b2): five 32-column streams. Software-pipelined:
    step s computes L1 of block s and L2 of block s-1, so L2's semaphore
    waits (on h evacuation) are long satisfied.
  - Every matmul is a 32-column unit with identical PE tile config (128,32),
    issued round-robin over the four PE column groups, so four units stream
    concurrently at all times: 22 units/step = 5.5 PE rounds of N=512.
    The p4/g16 pair rotates between column groups (0,1) and (2,3) by step
    parity to balance group load; the round-robin start rotates to match.
  - Output leaves the device feature-major in bf16 (tolerance is 2e-2;
    measured error ~2.9e-3); the host transposes/casts back to f32.

All matmuls are bf16; PSUM accumulation fp32 (TRN2 requires fp32 PSUM).
"""

import sys

import numpy as np
import ml_dtypes

sys.path.insert(0, "/opt/trn_rl_repo")

import concourse.bass as bass
import concourse.mybir as mybir
import concourse.tile as tile
from concourse import bacc
from concourse.bass_utils import run_bass_kernel_spmd

BF16 = mybir.dt.bfloat16
F32 = mybir.dt.float32
BF16_NP = ml_dtypes.bfloat16

LIMBS = [[0, 1, 2], [3, 4, 5], [6, 7], [8, 9], [10, 11, 12], [13, 14, 15],
         [6, 7, 1, 2], [6, 7, 4, 5], [6, 7, 11, 12], [6, 7, 14, 15], [6, 7, 9],
         [14, 15, 11, 12], [1, 2, 4, 5], [14, 15, 4, 5], [11, 12, 4, 5],
         [10, 0], [13, 3]]
NG = 17          # groups
HID = 16         # hidden per group
B, T, NJ, C = 2048, 243, 16, 3
NF = NJ * C      # 48 input features per token
NCORES = 8
BC = B // NCORES           # batches per core
TC = BC * T                # tokens per core
S = TC // 2                # token pairs per core (2-set packing)
KX = 2 * (NF + 1)          # 98: two sets of (48 features + ones row)
NBLK = 512                 # token-pairs per block (psum free dim)
NB = (S + NBLK - 1) // NBLK   # 61 blocks (60x512 + 1x384)

# L2 stream order across PSUM quarters of the `op` bank: stream q covers
# GROUPS_L2[q], reading h of L1 pass PASS_OF_STREAM[q] from the prev block.
GROUPS_L2 = [(12, 4), (0, 4), (4, 4), (8, 4)]
PASS_OF_STREAM = [3, 0, 1, 2]


def _host_weights(W1, b1, W2, b2, idx):
    """Build stationary operands + evac bias vectors on the host.

    Returns (wsb [128, 704] bf16, bsb [128, 1] f32).
      wsb cols 0-511: L1 passes 0-3 ([98,128] each: rows 0-47 set A block,
        row 48 = set A b1, rows 49-96 set B block, row 97 = set B b1).
      wsb cols 512-639: L2 streams q=0..3 ([128,32] each).
      wsb cols 640-671: L1 p4 (group 16 hidden, [98,32], b1 on ones rows).
      wsb cols 672-703: L2 g16, even-source variant ([128,32]: h16 rows
        0-31, b2 on ones-row 64 of the hx SBUF tile).
      wsb cols 704-735: L2 g16, odd-source variant (h16 rows 64-95, b2 on
        ones-row 0) — the hx tile layout rotates with step parity.
      bsb col 0: b2 for the L2 psum bank (per-partition).
      bsb col 1: relu mask for the merged hx evacuation (0.0 on h16 rows,
        -1e30 on g16-out rows, so max() is relu or identity per partition).
    """
    W1 = np.asarray(W1, np.float32)
    b1 = np.asarray(b1, np.float32)
    W2 = np.asarray(W2, np.float32)
    b2 = np.asarray(b2, np.float32)
    idx = np.asarray(idx)

    # Scatter per-group [12, 16] W1 blocks into the 48-feature space.
    # Padded limb rows of W1 are already zero, so += handles duplicates.
    w1full = np.zeros((NF, NG * HID), np.float32)
    for g in range(NG):
        for j in range(4):
            r = int(idx[g, j]) * C
            w1full[r:r + C, g * HID:(g + 1) * HID] += W1[g, j * C:(j + 1) * C, :]
    b1flat = b1.reshape(NG * HID)

    wsb = np.zeros((128, 736), np.float32)
    for w in range(4):
        blk = w1full[:, 64 * w:64 * w + 64]            # [48, 64]
        bias = b1flat[64 * w:64 * w + 64]
        wsb[0:NF, 128 * w:128 * w + 64] = blk          # set A
        wsb[NF, 128 * w:128 * w + 64] = bias
        wsb[NF + 1:2 * NF + 1, 128 * w + 64:128 * w + 128] = blk   # set B
        wsb[2 * NF + 1, 128 * w + 64:128 * w + 128] = bias
    for q, (g0, ng) in enumerate(GROUPS_L2):
        col = 512 + 32 * q
        for j in range(ng):
            g = g0 + j
            wsb[16 * j:16 * j + 16, col + 3 * j:col + 3 * j + 3] = W2[g]
            wsb[64 + 16 * j:64 + 16 * j + 16,
                col + 12 + 3 * j:col + 12 + 3 * j + 3] = W2[g]
    wsb[0:NF, 640:656] = w1full[:, 256:272]            # p4 set A
    wsb[NF, 640:656] = b1flat[256:272]
    wsb[NF + 1:2 * NF + 1, 656:672] = w1full[:, 256:272]   # p4 set B
    wsb[2 * NF + 1, 656:672] = b1flat[256:272]
    wsb[0:16, 672:675] = W2[16]                        # g16 even-src: set A
    wsb[16:32, 675:678] = W2[16]                       # g16 even-src: set B
    wsb[64, 672:675] = b2[16]                          # b2 via hx ones-row
    wsb[64, 675:678] = b2[16]
    wsb[64:80, 704:707] = W2[16]                       # g16 odd-src: set A
    wsb[80:96, 707:710] = W2[16]                       # g16 odd-src: set B
    wsb[0, 704:707] = b2[16]
    wsb[0, 707:710] = b2[16]

    bsb = np.zeros((128, 2), np.float32)
    for q, (g0, ng) in enumerate(GROUPS_L2):
        v = b2[g0:g0 + ng].reshape(-1)                 # 12 values
        bsb[32 * q:32 * q + 12, 0] = v
        bsb[32 * q + 12:32 * q + 24, 0] = v
    bsb[32:64, 1] = -1e30
    bsb[96:128, 1] = -1e30

    return wsb.astype(BF16_NP), bsb


def _build_nc():
    nc = bacc.Bacc(
        "TRN2", target_bir_lowering=False, debug=False, num_devices=NCORES,
    )
    x2 = nc.dram_tensor("x2", [KX, S], BF16, kind="ExternalInput").ap()
    wsd = nc.dram_tensor("wsd", [128, 736], BF16, kind="ExternalInput").ap()
    bsd = nc.dram_tensor("bsd", [128, 2], F32, kind="ExternalInput").ap()
    # Device output, feature-major bf16: rows 0-127 = L2 psum bank layout
    # (quarter q rows 32q..32q+24 real), rows 128-135 = g16 out (+2 pad).
    outd = nc.dram_tensor("outd", [136, S], BF16, kind="ExternalOutput").ap()

    with tile.TileContext(nc) as tc:
        with (
            tc.tile_pool(name="singles", bufs=1) as singles,
            tc.tile_pool(name="xin", bufs=4) as xin,
            tc.tile_pool(name="hsb", bufs=2) as hsb,
            tc.tile_pool(name="hxsb", bufs=2) as hxsb,
            tc.tile_pool(name="osb", bufs=3) as osb,
            tc.tile_pool(name="hps", bufs=1, space="PSUM") as hps,
            tc.tile_pool(name="ops", bufs=1, space="PSUM") as opsp,
            tc.tile_pool(name="hxps", bufs=1, space="PSUM") as hxps,
        ):
            ws = singles.tile([128, 736], BF16)
            nc.sync.dma_start(ws, wsd)
            bs = singles.tile([128, 2], F32)
            nc.sync.dma_start(bs, bsd)

            h_prev = None       # (h01, h23) sbuf tiles of previous block
            hx_prev = None      # hx sbuf tile of previous block
            nb_prev = 0

            for s in range(NB + 1):
                cur = s if s < NB else None
                prev = s - 1 if s >= 1 else None
                # p4/g16 column groups rotate by parity to balance load;
                # the unit round-robin starts on the groups the previous
                # step's p4/g16 did NOT use.
                ord_ = [0, 1, 2, 3] if s % 2 == 0 else [2, 3, 0, 1]
                xg, gg = ord_[0], ord_[1]       # p4 / g16 column groups
                if cur is not None:
                    off = cur * NBLK
                    nb = min(NBLK, S - off)
                    if s % 2 == 0:
                        # one DMA covers this block and the next
                        ld = min(2 * NBLK, S - off)
                        xtd = xin.tile([KX, 2 * NBLK], BF16, tag="xt")
                        nc.sync.dma_start(xtd[:, :ld], x2[:, off:off + ld])
                        xt = xtd[:, 0:NBLK]
                    else:
                        xt = xtd[:, NBLK:2 * NBLK]

                # ---- PE: 32-col units, round-robin over column groups ----
                if cur is not None:
                    hp01 = hps.tile([128, 2, NBLK], F32, tag="hp01")
                    hp23 = hps.tile([128, 2, NBLK], F32, tag="hp23", bufs=2)
                    for w in range(4):
                        hpt = hp01 if w < 2 else hp23
                        for j in ord_:
                            nc.tensor.matmul(
                                hpt[32 * j:32 * j + 32, w % 2, :nb],
                                lhsT=ws[0:KX,
                                        128 * w + 32 * j:128 * w + 32 * j + 32],
                                rhs=xt[:, :nb],
                                start=True, stop=True,
                                tile_position=(0, 32 * j),
                            )
                if prev is not None:
                    p01, p23 = h_prev
                    rhs_of = [p23[:, 1], p01[:, 0], p01[:, 1], p23[:, 0]]
                    op = opsp.tile([128, NBLK], F32, tag="op")
                    for q in ord_:
                        nc.tensor.matmul(
                            op[32 * q:32 * q + 32, :nb_prev],
                            lhsT=ws[0:128, 512 + 32 * q:512 + 32 * q + 32],
                            rhs=rhs_of[q][:, :nb_prev],
                            start=True, stop=True,
                            tile_position=(0, 32 * q),
                        )
                hx = hxps.tile([128, NBLK], F32, tag="hx")
                if cur is not None:
                    nc.tensor.matmul(
                        hx[32 * xg:32 * xg + 32, :nb],
                        lhsT=ws[0:KX, 640:672],
                        rhs=xt[:, :nb], start=True, stop=True,
                        tile_position=(0, 32 * xg),
                    )
                if prev is not None:
                    gcol = 672 if prev % 2 == 0 else 704
                    nc.tensor.matmul(
                        hx[32 * gg:32 * gg + 32, :nb_prev],
                        lhsT=ws[0:128, gcol:gcol + 32],
                        rhs=hx_prev[0:128, :nb_prev], start=True, stop=True,
                        tile_position=(0, 32 * gg),
                    )

                # ---- evacuations ----
                if cur is not None:
                    h01s = hsb.tile([128, 2, NBLK], BF16, tag="h01")
                    h23s = hsb.tile([128, 2, NBLK], BF16, tag="h23")
                    # DVE (the busier engine) evacuates the pass pair that
                    # finishes first, so its long op starts early; ACT's
                    # relu covers the later pair, whose bank is
                    # double-buffered to break the write-after-read cycle.
                    nc.vector.tensor_scalar(
                        h01s[:, :, :nb], hp01[:, :, :nb], 0.0, None,
                        mybir.AluOpType.max,
                    )
                    nc.scalar.activation(
                        out=h23s[:, :, :nb], in_=hp23[:, :, :nb],
                        func=mybir.ActivationFunctionType.Relu,
                    )
                    hts = (h01s, h23s)
                else:
                    hts = None
                # hx sbuf tile mirrors the hx psum rows (no partition shift):
                # even step: h16 at 0-31, g16 out at 32-39, ones row 64,
                # zeros elsewhere; odd step: h16 at 64-95, g16 out at 96-103,
                # ones row 0. The ones row bakes b2[16] into the g16 matmul.
                hxs = hxsb.tile([128, NBLK], BF16, tag="hxs")
                if cur is not None and prev is not None and nb == nb_prev:
                    lo = 32 * xg        # h16 + g16-out rows are adjacent
                    nc.vector.tensor_scalar(
                        hxs[lo:lo + 64, :nb], hx[lo:lo + 64, :nb],
                        bs[lo:lo + 64, 1:2], None,
                        mybir.AluOpType.max,
                    )
                else:
                    if cur is not None:
                        nc.vector.tensor_scalar(
                            hxs[32 * xg:32 * xg + 32, :nb],
                            hx[32 * xg:32 * xg + 32, :nb],
                            0.0, None, mybir.AluOpType.max,
                        )
                    else:
                        nc.gpsimd.memset(hxs[32 * xg:32 * xg + 32, :], 0.0)
                    if prev is not None:
                        nc.vector.tensor_scalar(
                            hxs[32 * gg:32 * gg + 32, :nb_prev],
                            hx[32 * gg:32 * gg + 32, :nb_prev],
                            0.0, None, mybir.AluOpType.add,
                        )
                    else:
                        nc.gpsimd.memset(hxs[32 * gg:32 * gg + 32, :], 0.0)
                if s % 2 == 0:
                    nc.gpsimd.memset(hxs[64:128, :], 0.0)
                    nc.gpsimd.memset(hxs[64:65, :], 1.0)
                else:
                    nc.gpsimd.memset(hxs[0:64, :], 0.0)
                    nc.gpsimd.memset(hxs[0:1, :], 1.0)
                if prev is not None:
                    ost = osb.tile([128, NBLK], BF16, tag="os")
                    nc.scalar.activation(
                        out=ost[:, :nb_prev], in_=op[:, :nb_prev],
                        func=mybir.ActivationFunctionType.Identity,
                        bias=bs[:, 0:1], scale=1.0,
                    )
                    offp = prev * NBLK
                    nc.scalar.dma_start(
                        outd[0:128, offp:offp + nb_prev], ost[:, :nb_prev])
                    nc.sync.dma_start(
                        outd[128:136, offp:offp + nb_prev],
                        hxs[32 * gg:32 * gg + 8, :nb_prev])

                h_prev = hts
                hx_prev = hxs
                if cur is not None:
                    nb_prev = nb
    nc.finalize()
    return nc


_NC_CACHE = None


def _get_nc():
    global _NC_CACHE
    if _NC_CACHE is None:
        _NC_CACHE = _build_nc()
    return _NC_CACHE


# outd row map (see _build_nc): stream q of GROUPS_L2 at rows 32q..32q+24
# (12 set-A cols then 12 set-B), g16 at rows 128-133.  In group order 0..16:
_ROWS_A = np.r_[32:44, 64:76, 96:108, 0:12, 128:131]
_ROWS_B = np.r_[44:56, 76:88, 108:120, 12:24, 131:134]


def _kernel_impl(x, W1, b1, W2, b2, idx, _want_trace=False):
    x = np.asarray(x, np.float32)
    wsb, bsb = _host_weights(W1, b1, W2, b2, idx)

    in_maps = []
    for c in range(NCORES):
        xc = x[c * BC:(c + 1) * BC].reshape(TC, NF)
        xt2 = np.empty((KX, S), BF16_NP)
        xt2[0:NF] = np.ascontiguousarray(xc[:S].T)
        xt2[NF] = np.float32(1.0)
        xt2[NF + 1:2 * NF + 1] = np.ascontiguousarray(xc[S:].T)
        xt2[2 * NF + 1] = np.float32(1.0)
        in_maps.append({"x2": xt2, "wsd": wsb, "bsd": bsb})

    nc = _get_nc()
    res = run_bass_kernel_spmd(
        nc, in_maps, core_ids=list(range(NCORES)), trace=_want_trace,
    )

    out = np.empty((B, T, NG, C), np.float32)
    for c in range(NCORES):
        od = np.asarray(res.results[c]["outd"], dtype=np.float32)  # [136, S]
        oc = np.empty((TC, NG * C), np.float32)
        oc[:S] = od[_ROWS_A].T
        oc[S:] = od[_ROWS_B].T
        out[c * BC:(c + 1) * BC] = oc.reshape(BC, T, NG, C)
    return out, res


def kernel(**inputs):
    out, _ = _kernel_impl(**inputs)
    return out


# revision 36
# speedup vs baseline: 1.0198x; 1.0198x over previous
"""Trainium2 Bass kernel for nn_BoneRefusion (17-group BoneMLP over [B,T,16,3]).

Strategy (pure data parallel over batch, 8 cores):
  - Host pre-packs per-core inputs feature-major in a 2-set layout:
      x2 [98, S] bf16, S = tokens_per_core/2.
      Rows 0-47 = 48 features (16 bones x 3 coords) of token set A (first
      half), row 48 = ones (bakes b1 into layer 1), rows 49-96 = set B,
      row 97 = ones. Column j holds the token pair (A_j, B_j).
  - Layer 1 (h = relu(x @ W1 + b1)): 17 column-unit matmuls of 32 PE columns
    each (4 passes x 4 units + one unit for group 16's hidden).
  - Layer 2 (out = h @ W2 + b2): five 32-column streams. Software-pipelined:
    step s computes L1 of block s and L2 of block s-1, so L2's semaphore
    waits (on h evacuation) are long satisfied.
  - Every matmul is a 32-column unit with identical PE tile config (128,32),
    issued round-robin over the four PE column groups, so four units stream
    concurrently at all times: 22 units/step = 5.5 PE rounds of N=512.
    The p4/g16 pair rotates between column groups (0,1) and (2,3) by step
    parity to balance group load; the round-robin start rotates to match.
  - Output leaves the device feature-major in bf16 (tolerance is 2e-2;
    measured error ~2.9e-3); the host transposes/casts back to f32.

All matmuls are bf16; PSUM accumulation fp32 (TRN2 requires fp32 PSUM).
"""

import sys

import numpy as np
import ml_dtypes

sys.path.insert(0, "/opt/trn_rl_repo")

import concourse.bass as bass
import concourse.mybir as mybir
import concourse.tile as tile
from concourse import bacc
from concourse.bass_utils import run_bass_kernel_spmd

BF16 = mybir.dt.bfloat16
F32 = mybir.dt.float32
BF16_NP = ml_dtypes.bfloat16

LIMBS = [[0, 1, 2], [3, 4, 5], [6, 7], [8, 9], [10, 11, 12], [13, 14, 15],
         [6, 7, 1, 2], [6, 7, 4, 5], [6, 7, 11, 12], [6, 7, 14, 15], [6, 7, 9],
         [14, 15, 11, 12], [1, 2, 4, 5], [14, 15, 4, 5], [11, 12, 4, 5],
         [10, 0], [13, 3]]
NG = 17          # groups
HID = 16         # hidden per group
B, T, NJ, C = 2048, 243, 16, 3
NF = NJ * C      # 48 input features per token
NCORES = 8
BC = B // NCORES           # batches per core
TC = BC * T                # tokens per core
S = TC // 2                # token pairs per core (2-set packing)
KX = 2 * (NF + 1)          # 98: two sets of (48 features + ones row)
NBLK = 512                 # token-pairs per block (psum free dim)
NB = (S + NBLK - 1) // NBLK   # 61 blocks (60x512 + 1x384)

# L2 stream order across PSUM quarters of the `op` bank: stream q covers
# GROUPS_L2[q], reading h of L1 pass PASS_OF_STREAM[q] from the prev block.
GROUPS_L2 = [(12, 4), (0, 4), (4, 4), (8, 4)]
PASS_OF_STREAM = [3, 0, 1, 2]


def _host_weights(W1, b1, W2, b2, idx):
    """Build stationary operands + evac bias vectors on the host.

    Returns (wsb [128, 704] bf16, bsb [128, 1] f32).
      wsb cols 0-511: L1 passes 0-3 ([98,128] each: rows 0-47 set A block,
        row 48 = set A b1, rows 49-96 set B block, row 97 = set B b1).
      wsb cols 512-639: L2 streams q=0..3 ([128,32] each).
      wsb cols 640-671: L1 p4 (group 16 hidden, [98,32], b1 on ones rows).
      wsb cols 672-703: L2 g16, even-source variant ([128,32]: h16 rows
        0-31, b2 on ones-row 64 of the hx SBUF tile).
      wsb cols 704-735: L2 g16, odd-source variant (h16 rows 64-95, b2 on
        ones-row 0) — the hx tile layout rotates with step parity.
      bsb col 0: b2 for the L2 psum bank (per-partition).
      bsb col 1: relu mask for the merged hx evacuation (0.0 on h16 rows,
        -1e30 on g16-out rows, so max() is relu or identity per partition).
    """
    W1 = np.asarray(W1, np.float32)
    b1 = np.asarray(b1, np.float32)
    W2 = np.asarray(W2, np.float32)
    b2 = np.asarray(b2, np.float32)
    idx = np.asarray(idx)

    # Scatter per-group [12, 16] W1 blocks into the 48-feature space.
    # Padded limb rows of W1 are already zero, so += handles duplicates.
    w1full = np.zeros((NF, NG * HID), np.float32)
    for g in range(NG):
        for j in range(4):
            r = int(idx[g, j]) * C
            w1full[r:r + C, g * HID:(g + 1) * HID] += W1[g, j * C:(j + 1) * C, :]
    b1flat = b1.reshape(NG * HID)

    wsb = np.zeros((128, 736), np.float32)
    for w in range(4):
        blk = w1full[:, 64 * w:64 * w + 64]            # [48, 64]
        bias = b1flat[64 * w:64 * w + 64]
        wsb[0:NF, 128 * w:128 * w + 64] = blk          # set A
        wsb[NF, 128 * w:128 * w + 64] = bias
        wsb[NF + 1:2 * NF + 1, 128 * w + 64:128 * w + 128] = blk   # set B
        wsb[2 * NF + 1, 128 * w + 64:128 * w + 128] = bias
    for q, (g0, ng) in enumerate(GROUPS_L2):
        col = 512 + 32 * q
        for j in range(ng):
            g = g0 + j
            wsb[16 * j:16 * j + 16, col + 3 * j:col + 3 * j + 3] = W2[g]
            wsb[64 + 16 * j:64 + 16 * j + 16,
                col + 12 + 3 * j:col + 12 + 3 * j + 3] = W2[g]
    wsb[0:NF, 640:656] = w1full[:, 256:272]            # p4 set A
    wsb[NF, 640:656] = b1flat[256:272]
    wsb[NF + 1:2 * NF + 1, 656:672] = w1full[:, 256:272]   # p4 set B
    wsb[2 * NF + 1, 656:672] = b1flat[256:272]
    wsb[0:16, 672:675] = W2[16]                        # g16 even-src: set A
    wsb[16:32, 675:678] = W2[16]                       # g16 even-src: set B
    wsb[64, 672:675] = b2[16]                          # b2 via hx ones-row
    wsb[64, 675:678] = b2[16]
    wsb[64:80, 704:707] = W2[16]                       # g16 odd-src: set A
    wsb[80:96, 707:710] = W2[16]                       # g16 odd-src: set B
    wsb[0, 704:707] = b2[16]
    wsb[0, 707:710] = b2[16]

    bsb = np.zeros((128, 2), np.float32)
    for q, (g0, ng) in enumerate(GROUPS_L2):
        v = b2[g0:g0 + ng].reshape(-1)                 # 12 values
        bsb[32 * q:32 * q + 12, 0] = v
        bsb[32 * q + 12:32 * q + 24, 0] = v
    bsb[32:64, 1] = -1e30
    bsb[96:128, 1] = -1e30

    return wsb.astype(BF16_NP), bsb


def _build_nc():
    nc = bacc.Bacc(
        "TRN2", target_bir_lowering=False, debug=False, num_devices=NCORES,
    )
    x2 = nc.dram_tensor("x2", [KX, S], BF16, kind="ExternalInput").ap()
    wsd = nc.dram_tensor("wsd", [128, 736], BF16, kind="ExternalInput").ap()
    bsd = nc.dram_tensor("bsd", [128, 2], F32, kind="ExternalInput").ap()
    # Device output, feature-major bf16: rows 0-127 = L2 psum bank layout
    # (quarter q rows 32q..32q+24 real), rows 128-135 = g16 out (+2 pad).
    outd = nc.dram_tensor("outd", [136, S], BF16, kind="ExternalOutput").ap()

    with tile.TileContext(nc) as tc:
        with (
            tc.tile_pool(name="singles", bufs=1) as singles,
            tc.tile_pool(name="xin", bufs=4) as xin,
            tc.tile_pool(name="hsb", bufs=2) as hsb,
            tc.tile_pool(name="hxsb", bufs=2) as hxsb,
            tc.tile_pool(name="osb", bufs=3) as osb,
            tc.tile_pool(name="hps", bufs=1, space="PSUM") as hps,
            tc.tile_pool(name="ops", bufs=1, space="PSUM") as opsp,
            tc.tile_pool(name="hxps", bufs=1, space="PSUM") as hxps,
        ):
            ws = singles.tile([128, 736], BF16)
            nc.sync.dma_start(ws, wsd)
            bs = singles.tile([128, 2], F32)
            nc.sync.dma_start(bs, bsd)

            h_prev = None       # (h01, h23) sbuf tiles of previous block
            hx_prev = None      # hx sbuf tile of previous block
            nb_prev = 0

            for s in range(NB + 1):
                cur = s if s < NB else None
                prev = s - 1 if s >= 1 else None
                # p4/g16 column groups rotate by parity to balance load;
                # the unit round-robin starts on the groups the previous
                # step's p4/g16 did NOT use.
                ord_ = [0, 1, 2, 3] if s % 2 == 0 else [2, 3, 0, 1]
                xg, gg = ord_[0], ord_[1]       # p4 / g16 column groups
                if cur is not None:
                    off = cur * NBLK
                    nb = min(NBLK, S - off)
                    if s % 2 == 0:
                        # one DMA covers this block and the next
                        ld = min(2 * NBLK, S - off)
                        xtd = xin.tile([KX, 2 * NBLK], BF16, tag="xt")
                        nc.sync.dma_start(xtd[:, :ld], x2[:, off:off + ld])
                        xt = xtd[:, 0:NBLK]
                    else:
                        xt = xtd[:, NBLK:2 * NBLK]

                # ---- PE: 32-col units, round-robin over column groups ----
                if cur is not None:
                    hp01 = hps.tile([128, 2, NBLK], F32, tag="hp01")
                    hp23 = hps.tile([128, 2, NBLK], F32, tag="hp23")
                    for w in range(4):
                        hpt = hp01 if w < 2 else hp23
                        for j in ord_:
                            nc.tensor.matmul(
                                hpt[32 * j:32 * j + 32, w % 2, :nb],
                                lhsT=ws[0:KX,
                                        128 * w + 32 * j:128 * w + 32 * j + 32],
                                rhs=xt[:, :nb],
                                start=True, stop=True,
                                tile_position=(0, 32 * j),
                            )
                if prev is not None:
                    p01, p23 = h_prev
                    rhs_of = [p23[:, 1], p01[:, 0], p01[:, 1], p23[:, 0]]
                    op = opsp.tile([128, NBLK], F32, tag="op", bufs=2)
                    for q in ord_:
                        nc.tensor.matmul(
                            op[32 * q:32 * q + 32, :nb_prev],
                            lhsT=ws[0:128, 512 + 32 * q:512 + 32 * q + 32],
                            rhs=rhs_of[q][:, :nb_prev],
                            start=True, stop=True,
                            tile_position=(0, 32 * q),
                        )
                hx = hxps.tile([128, NBLK], F32, tag="hx", bufs=2)
                if cur is not None:
                    nc.tensor.matmul(
                        hx[32 * xg:32 * xg + 32, :nb],
                        lhsT=ws[0:KX, 640:672],
                        rhs=xt[:, :nb], start=True, stop=True,
                        tile_position=(0, 32 * xg),
                    )
                if prev is not None:
                    gcol = 672 if prev % 2 == 0 else 704
                    nc.tensor.matmul(
                        hx[32 * gg:32 * gg + 32, :nb_prev],
                        lhsT=ws[0:128, gcol:gcol + 32],
                        rhs=hx_prev[0:128, :nb_prev], start=True, stop=True,
                        tile_position=(0, 32 * gg),
                    )

                # ---- evacuations ----
                if cur is not None:
                    h01s = hsb.tile([128, 2, NBLK], BF16, tag="h01")
                    h23s = hsb.tile([128, 2, NBLK], BF16, tag="h23")
                    # DVE (the busier engine) evacuates the pass pair that
                    # finishes first, so its long op starts early; ACT's
                    # relu covers the later pair, whose bank is
                    # double-buffered to break the write-after-read cycle.
                    nc.vector.tensor_scalar(
                        h01s[:, :, :nb], hp01[:, :, :nb], 0.0, None,
                        mybir.AluOpType.max,
                    )
                    nc.scalar.activation(
                        out=h23s[:, :, :nb], in_=hp23[:, :, :nb],
                        func=mybir.ActivationFunctionType.Relu,
                    )
                    hts = (h01s, h23s)
                else:
                    hts = None
                # hx sbuf tile mirrors the hx psum rows (no partition shift):
                # even step: h16 at 0-31, g16 out at 32-39, ones row 64,
                # zeros elsewhere; odd step: h16 at 64-95, g16 out at 96-103,
                # ones row 0. The ones row bakes b2[16] into the g16 matmul.
                hxs = hxsb.tile([128, NBLK], BF16, tag="hxs")
                if cur is not None and prev is not None and nb == nb_prev:
                    lo = 32 * xg        # h16 + g16-out rows are adjacent
                    nc.vector.tensor_scalar(
                        hxs[lo:lo + 64, :nb], hx[lo:lo + 64, :nb],
                        bs[lo:lo + 64, 1:2], None,
                        mybir.AluOpType.max,
                    )
                else:
                    if cur is not None:
                        nc.vector.tensor_scalar(
                            hxs[32 * xg:32 * xg + 32, :nb],
                            hx[32 * xg:32 * xg + 32, :nb],
                            0.0, None, mybir.AluOpType.max,
                        )
                    else:
                        nc.gpsimd.memset(hxs[32 * xg:32 * xg + 32, :], 0.0)
                    if prev is not None:
                        nc.vector.tensor_scalar(
                            hxs[32 * gg:32 * gg + 32, :nb_prev],
                            hx[32 * gg:32 * gg + 32, :nb_prev],
                            0.0, None, mybir.AluOpType.add,
                        )
                    else:
                        nc.gpsimd.memset(hxs[32 * gg:32 * gg + 32, :], 0.0)
                if s % 2 == 0:
                    nc.gpsimd.memset(hxs[64:128, :], 0.0)
                    nc.gpsimd.memset(hxs[64:65, :], 1.0)
                else:
                    nc.gpsimd.memset(hxs[0:64, :], 0.0)
                    nc.gpsimd.memset(hxs[0:1, :], 1.0)
                if prev is not None:
                    ost = osb.tile([128, NBLK], BF16, tag="os")
                    nc.scalar.activation(
                        out=ost[:, :nb_prev], in_=op[:, :nb_prev],
                        func=mybir.ActivationFunctionType.Identity,
                        bias=bs[:, 0:1], scale=1.0,
                    )
                    offp = prev * NBLK
                    nc.scalar.dma_start(
                        outd[0:128, offp:offp + nb_prev], ost[:, :nb_prev])
                    nc.sync.dma_start(
                        outd[128:136, offp:offp + nb_prev],
                        hxs[32 * gg:32 * gg + 8, :nb_prev])

                h_prev = hts
                hx_prev = hxs
                if cur is not None:
                    nb_prev = nb
    nc.finalize()
    return nc


_NC_CACHE = None


def _get_nc():
    global _NC_CACHE
    if _NC_CACHE is None:
        _NC_CACHE = _build_nc()
    return _NC_CACHE


# outd row map (see _build_nc): stream q of GROUPS_L2 at rows 32q..32q+24
# (12 set-A cols then 12 set-B), g16 at rows 128-133.  In group order 0..16:
_ROWS_A = np.r_[32:44, 64:76, 96:108, 0:12, 128:131]
_ROWS_B = np.r_[44:56, 76:88, 108:120, 12:24, 131:134]


def _kernel_impl(x, W1, b1, W2, b2, idx, _want_trace=False):
    x = np.asarray(x, np.float32)
    wsb, bsb = _host_weights(W1, b1, W2, b2, idx)

    in_maps = []
    for c in range(NCORES):
        xc = x[c * BC:(c + 1) * BC].reshape(TC, NF)
        xt2 = np.empty((KX, S), BF16_NP)
        xt2[0:NF] = np.ascontiguousarray(xc[:S].T)
        xt2[NF] = np.float32(1.0)
        xt2[NF + 1:2 * NF + 1] = np.ascontiguousarray(xc[S:].T)
        xt2[2 * NF + 1] = np.float32(1.0)
        in_maps.append({"x2": xt2, "wsd": wsb, "bsd": bsb})

    nc = _get_nc()
    res = run_bass_kernel_spmd(
        nc, in_maps, core_ids=list(range(NCORES)), trace=_want_trace,
    )

    out = np.empty((B, T, NG, C), np.float32)
    for c in range(NCORES):
        od = np.asarray(res.results[c]["outd"], dtype=np.float32)  # [136, S]
        oc = np.empty((TC, NG * C), np.float32)
        oc[:S] = od[_ROWS_A].T
        oc[S:] = od[_ROWS_B].T
        out[c * BC:(c + 1) * BC] = oc.reshape(BC, T, NG, C)
    return out, res


def kernel(**inputs):
    out, _ = _kernel_impl(**inputs)
    return out
